# revision 21
# baseline (speedup 1.0000x reference)
"""Canny filter (blur -> sobel -> orientation-quantized NMS) on 8 Trainium2 cores.

Self-contained: batch 16 is sharded 2 images/core (pure data parallel);
each core runs an identical Bass/Tile program on its [2,3,512,512] slice.

Both images are processed side by side in the free dimension of every tile
([128 rows x 2*(512+2) cols]). Per image, 5 overlapping 128-row tiles (valid
output rows 3..124 of each). Convolutions run on the TensorEngine as banded
matmuls (vertical taps in the 128x128 weight matrix, horizontal taps as
free-dim-shifted rhs accesses). NMS neighbor rows come from SBUF->SBUF DMA
row-shifted copies of msq; the orientation class uses fused
scalar_tensor_tensor compares on (Gx^2, Gy^2) and copy_predicated chains.
"""
import sys
import numpy as np

sys.path.insert(0, "/opt/trn_rl_repo")

import concourse.bacc as bacc
import concourse.tile as tile
from concourse import mybir
from concourse.bass_utils import run_bass_kernel_spmd
from contextlib import ExitStack

F32 = mybir.dt.float32
U8 = mybir.dt.uint8

B, C, H, W = 16, 3, 512, 512
N_CORES = 8
B_PER = B // N_CORES          # 2 images per core
P = 128                       # partitions per tile
WP = W + 2                    # padded width per image
FW = B_PER * W                # 1024: free width of unpadded working tiles
FWP = B_PER * WP              # 1028: free width of padded tiles
# row-tile input origins per image; tile t covers input rows [R, R+128),
# valid output rows are [R+3, R+125)
R_INS = [-3, 119, 241, 363, 387]

_ALU = mybir.AluOpType
_ACTF = mybir.ActivationFunctionType


def _banded(diag_vals, fold_top=False, fold_bot=False):
    """lhsT[k, m] for out[m] = sum_dk w[dk] * in[m+dk], dk in {-1,0,1}."""
    wm1, w0, wp1 = diag_vals
    A = np.zeros((P, P), np.float64)
    for m in range(P):
        for dk, wv in ((-1, wm1), (0, w0), (1, wp1)):
            k = m + dk
            if 0 <= k < P and wv != 0.0:
                A[k, m] += wv
    if fold_top:      # in[2] := in[3] for out row 3 (image-top replication)
        A[2, 3] = 0.0
        A[3, 3] = w0 + wm1
    if fold_bot:      # in[125] := in[124] for out row 124
        A[125, 124] = 0.0
        A[124, 124] = w0 + wp1
    return A.astype(np.float32)


def _build_weights():
    v = np.array([np.exp(-0.5), 1.0, np.exp(-0.5)], np.float64)
    sv = v.sum()
    g1 = v / sv                      # vertical gaussian taps
    h = v / (3.0 * sv)               # horizontal gaussian taps (folds the /C)
    ws = {}
    ws["Vg"] = _banded((g1[0], g1[1], g1[2]))
    ws["Ih0"] = (np.eye(P) * h[0]).astype(np.float32)
    ws["Ih1"] = (np.eye(P) * h[1]).astype(np.float32)
    # sobel x2 (integer taps): Gx = [1,2,1]_v (x) [-1,0,1]_h,
    # Gy = [-1,0,1]_v (x) [1,2,1]_h   (vertical +1 tap = the row below)
    for suf, kw in (("", {}), ("_t", {"fold_top": True}), ("_b", {"fold_bot": True})):
        ws["Vs" + suf] = _banded((1.0, 2.0, 1.0), **kw)
        ws["Vsn" + suf] = -ws["Vs" + suf]
        ws["Vd" + suf] = _banded((-1.0, 0.0, 1.0), **kw)
        ws["Vd2" + suf] = 2.0 * ws["Vd" + suf]
    return ws

# angle-band thresholds: tan((2j-1)*pi/16)^2, j=1..4
_TJ2 = [float(np.tan((2 * j - 1) * np.pi / 16.0) ** 2) for j in (1, 2, 3, 4)]

_NC_CACHE = None


def _build_program():
    nc = bacc.Bacc("TRN2", target_bir_lowering=False, debug=False, num_devices=N_CORES)
    img = nc.declare_dram_parameter("img", [B_PER, C, H, W], F32, isOutput=False)
    out = nc.declare_dram_parameter("out", [B_PER, 1, H, W], F32, isOutput=True)

    wnp = _build_weights()
    wkeys = sorted(wnp.keys())
    wcat = np.concatenate([wnp[k] for k in wkeys], axis=1)   # [128, 19*128]
    wdram_all = nc.inline_tensor(wcat, name="w_all")

    with tile.TileContext(nc) as tc, ExitStack() as ctx:
        cpool = ctx.enter_context(tc.tile_pool(name="consts", bufs=1))
        ctpool = ctx.enter_context(tc.tile_pool(name="cts", bufs=3))
        pool = ctx.enter_context(tc.tile_pool(name="work", bufs=2))
        mpool = ctx.enter_context(tc.tile_pool(name="masks", bufs=2))
        pp = ctx.enter_context(tc.tile_pool(name="ps", bufs=1, space="PSUM"))

        wall = cpool.tile([P, len(wkeys) * P], F32, tag="w_all")
        nc.sync.dma_start(wall[:], wdram_all[:])
        wsb = {k: wall[:, j * P:(j + 1) * P] for j, k in enumerate(wkeys)}
        zero = cpool.tile([P, FWP], F32, tag="zero")
        nc.vector.memset(zero[:], 0.0)

        # persistent ping-pong buffers: pads/edges zeroed once
        msqs, nsbs, ssbs = [], [], []
        for j in range(2):
            mq = cpool.tile([P, FWP], F32, tag=f"msq{j}")
            mv = mq[:, :].rearrange("p (i w) -> p i w", i=B_PER)
            nc.vector.memset(mv[:, :, 0:WP:WP - 1], 0.0)   # cols 0,513 per image
            msqs.append(mq)
            nsb = cpool.tile([P, FWP], F32, tag=f"nsb{j}")
            nc.vector.memset(nsb[0:32, :], 0.0)     # row 0 never DMA-written
            nsbs.append(nsb)
            ssb = cpool.tile([P, FWP], F32, tag=f"ssb{j}")
            nc.vector.memset(ssb[96:128, :], 0.0)   # row 127 never DMA-written
            ssbs.append(ssb)

        def im3(t):
            return t[:, :].rearrange("p (i w) -> p i w", i=B_PER)

        def shifted(base, off):
            # 2D-free AP: both images' [off, off+512) windows of a padded tile
            return im3(base)[:, :, off:off + W]

        def stage_load(t_i, R):
            # channel-summed load: base DMA carries channel 0, then two SWDGE
            # accumulate-DMAs add channels 1 and 2 in the DMA datapath.
            top = t_i == 0
            bot = t_i == len(R_INS) - 1
            ct = ctpool.tile([P, FW], F32, tag="ct")
            cv = ct[:, :].rearrange("p (i w) -> p i w", i=B_PER)
            if top:
                for i in range(B_PER):
                    nc.sync.dma_start(cv[0:3, i, :],
                                      img[i, 0, 0:1, :].broadcast_to((3, W)))
                    for k in (1, 2):
                        nc.gpsimd.dma_start(cv[0:3, i, :],
                                            img[i, k, 0:1, :].broadcast_to((3, W)),
                                            accum_op=_ALU.add)
                nc.sync.dma_start(cv[3:128, :, :],
                                  img[:, 0, 0:125, :].rearrange("i p w -> p i w"))
                for k in (1, 2):
                    nc.gpsimd.dma_start(cv[3:128, :, :],
                                        img[:, k, 0:125, :].rearrange("i p w -> p i w"),
                                        accum_op=_ALU.add)
            elif bot:
                nc.sync.dma_start(cv[0:125, :, :],
                                  img[:, 0, R:R + 125, :].rearrange("i p w -> p i w"))
                for k in (1, 2):
                    nc.gpsimd.dma_start(cv[0:125, :, :],
                                        img[:, k, R:R + 125, :].rearrange("i p w -> p i w"),
                                        accum_op=_ALU.add)
                for i in range(B_PER):
                    nc.sync.dma_start(cv[125:128, i, :],
                                      img[i, 0, 511:512, :].broadcast_to((3, W)))
                    for k in (1, 2):
                        nc.gpsimd.dma_start(cv[125:128, i, :],
                                            img[i, k, 511:512, :].broadcast_to((3, W)),
                                            accum_op=_ALU.add)
            else:
                nc.sync.dma_start(cv[:, :, :],
                                  img[:, 0, R:R + 128, :].rearrange("i p w -> p i w"))
                for k in (1, 2):
                    nc.gpsimd.dma_start(cv[:, :, :],
                                        img[:, k, R:R + 128, :].rearrange("i p w -> p i w"),
                                        accum_op=_ALU.add)
            return cv

        def stage_a(t_i, R, cv):
            top = t_i == 0
            bot = t_i == len(R_INS) - 1
            suf = "_t" if top else ("_b" if bot else "")

            # ---- vertical gauss on the channel-summed tile (PE)
            ps_u = pp.tile([P, FW], F32, tag="u")
            for i in range(B_PER):
                nc.tensor.matmul(ps_u[:, i * W:(i + 1) * W], wsb["Vg"],
                                 cv[:, i, :], start=True, stop=True)
            u = pool.tile([P, FWP], F32, tag="u_sb")
            nc.scalar.copy(shifted(u, 1), im3(ps_u))
            nc.vector.tensor_copy(im3(u)[:, :, 0:WP:WP - 1], im3(u)[:, :, 1:WP:W - 1])

            # ---- horizontal gauss (PE, identity-scaled shifted rhs)
            ps_t = pp.tile([P, FW], F32, tag="t")
            for i in range(B_PER):
                o = i * WP
                s_ = slice(i * W, (i + 1) * W)
                nc.tensor.matmul(ps_t[:, s_], wsb["Ih0"], u[:, o:o + W],
                                 start=True, stop=False)
                nc.tensor.matmul(ps_t[:, s_], wsb["Ih1"], u[:, o + 1:o + 1 + W],
                                 start=False, stop=False)
                nc.tensor.matmul(ps_t[:, s_], wsb["Ih0"], u[:, o + 2:o + 2 + W],
                                 start=False, stop=True)
            tt = pool.tile([P, FWP], F32, tag="t_sb")
            nc.scalar.copy(shifted(tt, 1), im3(ps_t))
            nc.vector.tensor_copy(im3(tt)[:, :, 0:WP:WP - 1], im3(tt)[:, :, 1:WP:W - 1])

            # ---- sobel (PE)
            ps_gx = pp.tile([P, FW], F32, tag="gx")
            ps_gy = pp.tile([P, FW], F32, tag="gy")
            for i in range(B_PER):
                o = i * WP
                s_ = slice(i * W, (i + 1) * W)
                nc.tensor.matmul(ps_gx[:, s_], wsb["Vsn" + suf], tt[:, o:o + W],
                                 start=True, stop=False)
                nc.tensor.matmul(ps_gx[:, s_], wsb["Vs" + suf], tt[:, o + 2:o + 2 + W],
                                 start=False, stop=True)
                nc.tensor.matmul(ps_gy[:, s_], wsb["Vd" + suf], tt[:, o:o + W],
                                 start=True, stop=False)
                nc.tensor.matmul(ps_gy[:, s_], wsb["Vd2" + suf], tt[:, o + 1:o + 1 + W],
                                 start=False, stop=False)
                nc.tensor.matmul(ps_gy[:, s_], wsb["Vd" + suf], tt[:, o + 2:o + 2 + W],
                                 start=False, stop=True)

            # ---- squares (ACT) + sign masks (DVE, straight from PSUM)
            sqx = pool.tile([P, FW], F32, tag="sqx")
            nc.scalar.activation(sqx[:], ps_gx[:], _ACTF.Square)
            sqy = pool.tile([P, FW], F32, tag="sqy")
            nc.scalar.activation(sqy[:], ps_gy[:], _ACTF.Square)
            gxpos = mpool.tile([P, FW], U8, tag="gxpos")
            nc.vector.tensor_scalar(gxpos[:], ps_gx[:], 0.0, None, _ALU.is_gt)
            s01 = mpool.tile([P, FW], U8, tag="s01")
            nc.vector.scalar_tensor_tensor(s01[:], ps_gy[:], 0.0, gxpos[:],
                                           _ALU.is_gt, _ALU.is_equal)

            msq = msqs[t_i % 2]
            mc = shifted(msq, 1)
            nc.vector.tensor_tensor(mc, sqx[:].rearrange("p (i w) -> p i w", i=B_PER),
                                    sqy[:].rearrange("p (i w) -> p i w", i=B_PER), _ALU.add)

            # ---- N/S row-shifted copies of msq (SBUF->SBUF DMA, pads incl.)
            nsb, ssb = nsbs[t_i % 2], ssbs[t_i % 2]
            nc.sync.dma_start(nsb[1:128, :], msq[0:127, :])
            nc.sync.dma_start(ssb[0:127, :], msq[1:128, :])
            if top:
                nc.sync.dma_start(nsb[3:4, :], zero[0:1, :])
            if bot:
                nc.sync.dma_start(ssb[124:125, :], zero[0:1, :])
            return dict(sqx=sqx, sqy=sqy, s01=s01, msq=msq, mc=mc, nsb=nsb, ssb=ssb, R=R)

        def stage_b(st):
            sqx, sqy, s01 = st["sqx"], st["sqy"], st["s01"]
            msq, mc, nsb, ssb, R = st["msq"], st["mc"], st["nsb"], st["ssb"], st["R"]

            cms = []
            for j, tj2 in enumerate(_TJ2):
                cm = mpool.tile([P, FW], U8, tag=f"c{j}m")
                nc.vector.scalar_tensor_tensor(cm[:], sqx[:], tj2, sqy[:],
                                               _ALU.mult, _ALU.is_lt)
                cms.append(cm)
            q = mpool.tile([P, FW], U8, tag="q")
            nc.vector.tensor_tensor(q[:], s01[:], cms[2][:], _ALU.not_equal)

            a1 = pool.tile([P, FW], F32, tag="a1")   # NE / SW
            nc.vector.tensor_tensor(im3(a1), shifted(nsb, 2), shifted(ssb, 0), _ALU.max)
            a3 = pool.tile([P, FW], F32, tag="a3")   # NW / SE
            nc.vector.tensor_tensor(im3(a3), shifted(nsb, 0), shifted(ssb, 2), _ALU.max)
            adiag = pool.tile([P, FW], F32, tag="adiag")
            nc.vector.tensor_copy(adiag[:], a3[:])
            nc.vector.copy_predicated(adiag[:], q[:], a1[:])
            a2 = pool.tile([P, FW], F32, tag="a2")   # N / S
            nc.vector.tensor_tensor(im3(a2), shifted(nsb, 1), shifted(ssb, 1), _ALU.max)
            m = pool.tile([P, FW], F32, tag="m")     # E / W
            nc.vector.tensor_tensor(im3(m), shifted(msq, 0), shifted(msq, 2), _ALU.max)
            a0c = pool.tile([P, FW], F32, tag="a0c")
            nc.vector.tensor_copy(a0c[:], m[:])
            nc.vector.copy_predicated(m[:], cms[0][:], adiag[:])
            nc.vector.copy_predicated(m[:], cms[1][:], a2[:])
            nc.vector.copy_predicated(m[:], cms[2][:], adiag[:])
            nc.vector.copy_predicated(m[:], cms[3][:], a0c[:])

            rm = mpool.tile([P, FW], U8, tag="rm")
            nc.vector.tensor_tensor(im3(rm), im3(m), mc, _ALU.is_ge)
            nc.vector.copy_predicated(mc, im3(rm), im3(zero[:, 0:FW]))
            osb = pool.tile([P, FW], F32, tag="osb")
            nc.scalar.activation(im3(osb), mc, _ACTF.Sqrt, scale=0.25)

            r0, r1 = R + 3, R + 125
            nc.sync.dma_start(
                out[:, 0, r0:r1, :].rearrange("i r w -> r i w"),
                osb[3:125, :].rearrange("p (i w) -> p i w", i=B_PER))

        # software-pipelined emission:
        #   load(k+2) and stage A(k+1) are emitted before stage B(k)
        n = len(R_INS)
        cvs = [None] * n
        cvs[0] = stage_load(0, R_INS[0])
        cvs[1] = stage_load(1, R_INS[1])
        pending = None
        for t_i, R in enumerate(R_INS):
            if t_i + 2 < n:
                cvs[t_i + 2] = stage_load(t_i + 2, R_INS[t_i + 2])
            st = stage_a(t_i, R, cvs[t_i])
            if pending is not None:
                stage_b(pending)
            pending = st
        stage_b(pending)

    nc.compile()
    return nc


def _get_program():
    global _NC_CACHE
    if _NC_CACHE is None:
        _NC_CACHE = _build_program()
    return _NC_CACHE


def kernel(img, w_gauss=None, w_sobel_x=None, w_sobel_y=None, w_dir=None):
    img = np.ascontiguousarray(np.asarray(img, dtype=np.float32))
    assert img.shape == (B, C, H, W)
    nc = _get_program()
    in_maps = [{"img": img[c * B_PER:(c + 1) * B_PER]} for c in range(N_CORES)]
    res = run_bass_kernel_spmd(nc, in_maps, list(range(N_CORES)))
    return np.concatenate([res.results[c]["out"] for c in range(N_CORES)], axis=0)


# revision 23
# speedup vs baseline: 1.1083x; 1.1083x over previous
"""Canny filter (blur -> sobel -> orientation-quantized NMS) on 8 Trainium2 cores.

Self-contained: batch 16 is sharded 2 images/core (pure data parallel);
each core runs an identical Bass/Tile program on its [2,3,512,512] slice.

Both images are processed side by side in the free dimension of every tile
([128 rows x 2*(512+2) cols]). Per image, 5 overlapping 128-row tiles (valid
output rows 3..124 of each). Convolutions run on the TensorEngine as banded
matmuls (vertical taps in the 128x128 weight matrix, horizontal taps as
free-dim-shifted rhs accesses). NMS neighbor rows come from SBUF->SBUF DMA
row-shifted copies of msq; the orientation class uses fused
scalar_tensor_tensor compares on (Gx^2, Gy^2) and copy_predicated chains.
"""
import sys
import numpy as np

sys.path.insert(0, "/opt/trn_rl_repo")

import concourse.bacc as bacc
import concourse.tile as tile
from concourse import mybir
from concourse.bass_utils import run_bass_kernel_spmd
from contextlib import ExitStack

F32 = mybir.dt.float32
U8 = mybir.dt.uint8

B, C, H, W = 16, 3, 512, 512
N_CORES = 8
B_PER = B // N_CORES          # 2 images per core
P = 128                       # partitions per tile
WP = W + 2                    # padded width per image
FW = B_PER * W                # 1024: free width of unpadded working tiles
FWP = B_PER * WP              # 1028: free width of padded tiles
# row-tile input origins per image; tile t covers input rows [R, R+128),
# valid output rows are [R+3, R+125)
R_INS = [-3, 119, 241, 363, 387]

_ALU = mybir.AluOpType
_ACTF = mybir.ActivationFunctionType


def _banded(diag_vals, fold_top=False, fold_bot=False):
    """lhsT[k, m] for out[m] = sum_dk w[dk] * in[m+dk], dk in {-1,0,1}."""
    wm1, w0, wp1 = diag_vals
    A = np.zeros((P, P), np.float64)
    for m in range(P):
        for dk, wv in ((-1, wm1), (0, w0), (1, wp1)):
            k = m + dk
            if 0 <= k < P and wv != 0.0:
                A[k, m] += wv
    if fold_top:      # in[2] := in[3] for out row 3 (image-top replication)
        A[2, 3] = 0.0
        A[3, 3] = w0 + wm1
    if fold_bot:      # in[125] := in[124] for out row 124
        A[125, 124] = 0.0
        A[124, 124] = w0 + wp1
    return A.astype(np.float32)


def _build_weights():
    v = np.array([np.exp(-0.5), 1.0, np.exp(-0.5)], np.float64)
    sv = v.sum()
    g1 = v / sv                      # vertical gaussian taps
    h = v / (3.0 * sv)               # horizontal gaussian taps (folds the /C)
    ws = {}
    ws["Vg"] = _banded((g1[0], g1[1], g1[2]))
    ws["Ih0"] = (np.eye(P) * h[0]).astype(np.float32)
    ws["Ih1"] = (np.eye(P) * h[1]).astype(np.float32)
    # sobel x2 (integer taps): Gx = [1,2,1]_v (x) [-1,0,1]_h,
    # Gy = [-1,0,1]_v (x) [1,2,1]_h   (vertical +1 tap = the row below)
    for suf, kw in (("", {}), ("_t", {"fold_top": True}), ("_b", {"fold_bot": True})):
        ws["Vs" + suf] = _banded((1.0, 2.0, 1.0), **kw)
        ws["Vsn" + suf] = -ws["Vs" + suf]
        ws["Vd" + suf] = _banded((-1.0, 0.0, 1.0), **kw)
        ws["Vd2" + suf] = 2.0 * ws["Vd" + suf]
    return ws

# angle-band thresholds: tan((2j-1)*pi/16)^2, j=1..4
_TJ2 = [float(np.tan((2 * j - 1) * np.pi / 16.0) ** 2) for j in (1, 2, 3, 4)]

_NC_CACHE = None


def _build_program():
    nc = bacc.Bacc("TRN2", target_bir_lowering=False, debug=False, num_devices=N_CORES)
    img = nc.declare_dram_parameter("img", [B_PER, C, H, W], F32, isOutput=False)
    out = nc.declare_dram_parameter("out", [B_PER, 1, H, W], F32, isOutput=True)

    wnp = _build_weights()
    wkeys = sorted(wnp.keys())
    wcat = np.concatenate([wnp[k] for k in wkeys], axis=1)   # [128, 19*128]
    wdram_all = nc.inline_tensor(wcat, name="w_all")

    with tile.TileContext(nc) as tc, ExitStack() as ctx:
        cpool = ctx.enter_context(tc.tile_pool(name="consts", bufs=1))
        ctpool = ctx.enter_context(tc.tile_pool(name="cts", bufs=3))
        pool = ctx.enter_context(tc.tile_pool(name="work", bufs=2))
        mpool = ctx.enter_context(tc.tile_pool(name="masks", bufs=2))
        pp = ctx.enter_context(tc.tile_pool(name="ps", bufs=1, space="PSUM"))

        wall = cpool.tile([P, len(wkeys) * P], F32, tag="w_all")
        nc.sync.dma_start(wall[:], wdram_all[:])
        wsb = {k: wall[:, j * P:(j + 1) * P] for j, k in enumerate(wkeys)}
        zero = cpool.tile([P, FWP], F32, tag="zero")
        nc.vector.memset(zero[:], 0.0)

        # persistent ping-pong buffers: pads/edges zeroed once
        msqs, nsbs, ssbs = [], [], []
        for j in range(2):
            mq = cpool.tile([P, FWP], F32, tag=f"msq{j}")
            mv = mq[:, :].rearrange("p (i w) -> p i w", i=B_PER)
            nc.vector.memset(mv[:, :, 0:WP:WP - 1], 0.0)   # cols 0,513 per image
            msqs.append(mq)
            nsb = cpool.tile([P, FWP], F32, tag=f"nsb{j}")
            nc.vector.memset(nsb[0:32, :], 0.0)     # row 0 never DMA-written
            nsbs.append(nsb)
            ssb = cpool.tile([P, FWP], F32, tag=f"ssb{j}")
            nc.vector.memset(ssb[96:128, :], 0.0)   # row 127 never DMA-written
            ssbs.append(ssb)

        def im3(t):
            return t[:, :].rearrange("p (i w) -> p i w", i=B_PER)

        def shifted(base, off):
            # 2D-free AP: both images' [off, off+512) windows of a padded tile
            return im3(base)[:, :, off:off + W]

        def stage_load(t_i, R):
            # channel-summed load: base DMA carries channel 0, then two SWDGE
            # accumulate-DMAs add channels 1 and 2 in the DMA datapath.
            top = t_i == 0
            bot = t_i == len(R_INS) - 1
            ct = ctpool.tile([P, FW], F32, tag="ct")
            cv = ct[:, :].rearrange("p (i w) -> p i w", i=B_PER)
            if top:
                for i in range(B_PER):
                    nc.sync.dma_start(cv[0:3, i, :],
                                      img[i, 0, 0:1, :].broadcast_to((3, W)))
                    for k in (1, 2):
                        nc.gpsimd.dma_start(cv[0:3, i, :],
                                            img[i, k, 0:1, :].broadcast_to((3, W)),
                                            accum_op=_ALU.add)
                nc.sync.dma_start(cv[3:128, :, :],
                                  img[:, 0, 0:125, :].rearrange("i p w -> p i w"))
                for k in (1, 2):
                    nc.gpsimd.dma_start(cv[3:128, :, :],
                                        img[:, k, 0:125, :].rearrange("i p w -> p i w"),
                                        accum_op=_ALU.add)
            elif bot:
                nc.sync.dma_start(cv[0:125, :, :],
                                  img[:, 0, R:R + 125, :].rearrange("i p w -> p i w"))
                for k in (1, 2):
                    nc.gpsimd.dma_start(cv[0:125, :, :],
                                        img[:, k, R:R + 125, :].rearrange("i p w -> p i w"),
                                        accum_op=_ALU.add)
                for i in range(B_PER):
                    nc.sync.dma_start(cv[125:128, i, :],
                                      img[i, 0, 511:512, :].broadcast_to((3, W)))
                    for k in (1, 2):
                        nc.gpsimd.dma_start(cv[125:128, i, :],
                                            img[i, k, 511:512, :].broadcast_to((3, W)),
                                            accum_op=_ALU.add)
            else:
                nc.sync.dma_start(cv[:, :, :],
                                  img[:, 0, R:R + 128, :].rearrange("i p w -> p i w"))
                for k in (1, 2):
                    nc.gpsimd.dma_start(cv[:, :, :],
                                        img[:, k, R:R + 128, :].rearrange("i p w -> p i w"),
                                        accum_op=_ALU.add)
            return cv

        def stage_a(t_i, R, cv):
            top = t_i == 0
            bot = t_i == len(R_INS) - 1
            suf = "_t" if top else ("_b" if bot else "")

            # ---- vertical gauss on the channel-summed tile (PE)
            ps_u = pp.tile([P, FW], F32, tag="u")
            for i in range(B_PER):
                nc.tensor.matmul(ps_u[:, i * W:(i + 1) * W], wsb["Vg"],
                                 cv[:, i, :], start=True, stop=True)
            u = pool.tile([P, FWP], F32, tag="u_sb")
            nc.scalar.copy(shifted(u, 1), im3(ps_u))
            nc.scalar.copy(im3(u)[:, :, 0:WP:WP - 1], im3(u)[:, :, 1:WP:W - 1])

            # ---- horizontal gauss (PE, identity-scaled shifted rhs)
            ps_t = pp.tile([P, FW], F32, tag="t")
            for i in range(B_PER):
                o = i * WP
                s_ = slice(i * W, (i + 1) * W)
                nc.tensor.matmul(ps_t[:, s_], wsb["Ih0"], u[:, o:o + W],
                                 start=True, stop=False)
                nc.tensor.matmul(ps_t[:, s_], wsb["Ih1"], u[:, o + 1:o + 1 + W],
                                 start=False, stop=False)
                nc.tensor.matmul(ps_t[:, s_], wsb["Ih0"], u[:, o + 2:o + 2 + W],
                                 start=False, stop=True)
            tt = pool.tile([P, FWP], F32, tag="t_sb")
            nc.scalar.copy(shifted(tt, 1), im3(ps_t))
            nc.scalar.copy(im3(tt)[:, :, 0:WP:WP - 1], im3(tt)[:, :, 1:WP:W - 1])

            # ---- sobel (PE)
            ps_gx = pp.tile([P, FW], F32, tag="gx")
            ps_gy = pp.tile([P, FW], F32, tag="gy")
            for i in range(B_PER):
                o = i * WP
                s_ = slice(i * W, (i + 1) * W)
                nc.tensor.matmul(ps_gx[:, s_], wsb["Vsn" + suf], tt[:, o:o + W],
                                 start=True, stop=False)
                nc.tensor.matmul(ps_gx[:, s_], wsb["Vs" + suf], tt[:, o + 2:o + 2 + W],
                                 start=False, stop=True)
                nc.tensor.matmul(ps_gy[:, s_], wsb["Vd" + suf], tt[:, o:o + W],
                                 start=True, stop=False)
                nc.tensor.matmul(ps_gy[:, s_], wsb["Vd2" + suf], tt[:, o + 1:o + 1 + W],
                                 start=False, stop=False)
                nc.tensor.matmul(ps_gy[:, s_], wsb["Vd" + suf], tt[:, o + 2:o + 2 + W],
                                 start=False, stop=True)

            # ---- squares (ACT) + sign masks (DVE, straight from PSUM)
            sqx = pool.tile([P, FW], F32, tag="sqx")
            nc.scalar.activation(sqx[:], ps_gx[:], _ACTF.Square)
            sqy = pool.tile([P, FW], F32, tag="sqy")
            nc.scalar.activation(sqy[:], ps_gy[:], _ACTF.Square)
            gxpos = mpool.tile([P, FW], U8, tag="gxpos")
            nc.vector.tensor_scalar(gxpos[:], ps_gx[:], 0.0, None, _ALU.is_gt)
            s01 = mpool.tile([P, FW], U8, tag="s01")
            nc.vector.scalar_tensor_tensor(s01[:], ps_gy[:], 0.0, gxpos[:],
                                           _ALU.is_gt, _ALU.is_equal)

            msq = msqs[t_i % 2]
            mc = shifted(msq, 1)
            nc.vector.tensor_tensor(mc, sqx[:].rearrange("p (i w) -> p i w", i=B_PER),
                                    sqy[:].rearrange("p (i w) -> p i w", i=B_PER), _ALU.add)

            # ---- N/S row-shifted copies of msq (SBUF->SBUF DMA, pads incl.)
            nsb, ssb = nsbs[t_i % 2], ssbs[t_i % 2]
            nc.sync.dma_start(nsb[1:128, :], msq[0:127, :])
            nc.sync.dma_start(ssb[0:127, :], msq[1:128, :])
            if top:
                nc.sync.dma_start(nsb[3:4, :], zero[0:1, :])
            if bot:
                nc.sync.dma_start(ssb[124:125, :], zero[0:1, :])
            return dict(sqx=sqx, sqy=sqy, s01=s01, msq=msq, mc=mc, nsb=nsb, ssb=ssb, R=R)

        def stage_b(st):
            sqx, sqy, s01 = st["sqx"], st["sqy"], st["s01"]
            msq, mc, nsb, ssb, R = st["msq"], st["mc"], st["nsb"], st["ssb"], st["R"]

            cms = []
            for j, tj2 in enumerate(_TJ2):
                cm = mpool.tile([P, FW], U8, tag=f"c{j}m")
                nc.vector.scalar_tensor_tensor(cm[:], sqx[:], tj2, sqy[:],
                                               _ALU.mult, _ALU.is_lt)
                cms.append(cm)
            q = mpool.tile([P, FW], U8, tag="q")
            nc.vector.tensor_tensor(q[:], s01[:], cms[2][:], _ALU.not_equal)

            a1 = pool.tile([P, FW], F32, tag="a1")   # NE / SW
            nc.vector.tensor_tensor(im3(a1), shifted(nsb, 2), shifted(ssb, 0), _ALU.max)
            a3 = pool.tile([P, FW], F32, tag="a3")   # NW / SE
            nc.vector.tensor_tensor(im3(a3), shifted(nsb, 0), shifted(ssb, 2), _ALU.max)
            adiag = pool.tile([P, FW], F32, tag="adiag")
            nc.scalar.copy(adiag[:], a3[:])
            nc.vector.copy_predicated(adiag[:], q[:], a1[:])
            a2 = pool.tile([P, FW], F32, tag="a2")   # N / S
            nc.vector.tensor_tensor(im3(a2), shifted(nsb, 1), shifted(ssb, 1), _ALU.max)
            m = pool.tile([P, FW], F32, tag="m")     # E / W
            nc.vector.tensor_tensor(im3(m), shifted(msq, 0), shifted(msq, 2), _ALU.max)
            a0c = pool.tile([P, FW], F32, tag="a0c")
            nc.scalar.copy(a0c[:], m[:])
            nc.vector.copy_predicated(m[:], cms[0][:], adiag[:])
            nc.vector.copy_predicated(m[:], cms[1][:], a2[:])
            nc.vector.copy_predicated(m[:], cms[2][:], adiag[:])
            nc.vector.copy_predicated(m[:], cms[3][:], a0c[:])

            rm = mpool.tile([P, FW], U8, tag="rm")
            nc.vector.tensor_tensor(im3(rm), im3(m), mc, _ALU.is_ge)
            nc.vector.copy_predicated(mc, im3(rm), im3(zero[:, 0:FW]))
            osb = pool.tile([P, FW], F32, tag="osb")
            nc.scalar.activation(im3(osb), mc, _ACTF.Sqrt, scale=0.25)

            r0, r1 = R + 3, R + 125
            nc.sync.dma_start(
                out[:, 0, r0:r1, :].rearrange("i r w -> r i w"),
                osb[3:125, :].rearrange("p (i w) -> p i w", i=B_PER))

        # software-pipelined emission:
        #   load(k+2) and stage A(k+1) are emitted before stage B(k)
        n = len(R_INS)
        cvs = [None] * n
        cvs[0] = stage_load(0, R_INS[0])
        cvs[1] = stage_load(1, R_INS[1])
        pending = None
        for t_i, R in enumerate(R_INS):
            if t_i + 2 < n:
                cvs[t_i + 2] = stage_load(t_i + 2, R_INS[t_i + 2])
            st = stage_a(t_i, R, cvs[t_i])
            if pending is not None:
                stage_b(pending)
            pending = st
        stage_b(pending)

    nc.compile()
    return nc


def _get_program():
    global _NC_CACHE
    if _NC_CACHE is None:
        _NC_CACHE = _build_program()
    return _NC_CACHE


def kernel(img, w_gauss=None, w_sobel_x=None, w_sobel_y=None, w_dir=None):
    img = np.ascontiguousarray(np.asarray(img, dtype=np.float32))
    assert img.shape == (B, C, H, W)
    nc = _get_program()
    in_maps = [{"img": img[c * B_PER:(c + 1) * B_PER]} for c in range(N_CORES)]
    res = run_bass_kernel_spmd(nc, in_maps, list(range(N_CORES)))
    return np.concatenate([res.results[c]["out"] for c in range(N_CORES)], axis=0)
